# revision 26
# baseline (speedup 1.0000x reference)
"""BottleneckMamba Trainium2 kernel (self-contained), v2.

out = x + cv2( scale * out_proj( LN(cross-merge(4-dir selective scan(N=1))) * z ) )

3 SPMD launches on 8 NeuronCores:
  L1 (core=(b, image-half)): cv1 -> h (bias on DVE); depthwise3x3*in_proj
     folded into 9 matmuls -> silu -> xc ; z = silu(Wz@h).
  L2 (core=(b, dir-group)): per direction: dtd = dt_w8 @ dts (rank-8 rows
     preshipped) -> exp/ln1p/exp on ACT (one table set) -> tbt = dt*v
     (v = u*B preshipped) -> tensor_tensor_scan -> mc = h*C (C broadcast by
     replicating DMA). Chunks processed ping-pong (reverse dir from the top,
     forward from the bottom) so both scans stream concurrently; m = mcf+mcr.
  L3 (core=(b, half)): y = m02 + m13^T + (sum_k D_k).xc; LayerNorm stats via
     ones-matmul per position chunk (no global barrier: stats are
     per-position), rstd/-mu*rstd rows written to DRAM and replicated back by
     DMA, normalize * z, fused (cv2 @ diag(scale) @ out_proj @ diag(ln_g))
     matmul -> bf16 delta.
Host: shards/reassembles, transposes between launches, computes the rank-8
dt rows + B/C rows + v = u*B from xc (bf16 data prep), adds residual x,
cv2 bias and the (zero here) ln_b term.
"""
import os
import sys

sys.path.insert(0, '/opt/trn_rl_repo')

import numpy as np
import ml_dtypes

import concourse.bass as bass
import concourse.tile as tile
import concourse.mybir as mybir
from concourse.bass_utils import run_bass_kernel_spmd

bf16 = mybir.dt.bfloat16
f32 = mybir.dt.float32
MULT, ADD = mybir.AluOpType.mult, mybir.AluOpType.add
SUB = mybir.AluOpType.subtract
AF = mybir.ActivationFunctionType
NBF = ml_dtypes.bfloat16

B, C1, C2, H, W = 4, 256, 256, 128, 128
Cm, K, R = 128, 4, 8
L = H * W          # 16384
HH = H // 2        # 64 rows per half
LH = HH * W        # 8192
CH = 2048          # L2 chunk
NCH = L // CH      # 8
CH3 = 1024         # L3 chunk
NC3 = LH // CH3    # 8

EXEC_TIMES = {}    # launch -> exec ns (MAMBA_TRACE=1)
TRACES = {}        # launch -> (insts, trace_path) (MAMBA_TRACE=1)
_CACHE = {}


def _split_multiwaits(nc):
    """walrus here accepts ONE sync-wait per instruction; hoist extras into
    single-wait same-engine NOPs inserted before the instruction."""
    for f in nc.m.functions:
        for bb in f.blocks:
            il = bb.instructions
            i = 0
            while i < len(il):
                ins = il[i]
                si = getattr(ins, "sync_info", None)
                if si is not None and len(si.on_wait) > 1:
                    waits = list(si.on_wait)
                    ins.sync_info = mybir.SyncInfo(
                        on_wait=[waits[-1]], on_update=list(si.on_update))
                    for w in waits[:-1]:
                        nop = mybir.InstNoOp(
                            name=nc.get_next_instruction_name(), ins=[], outs=[])
                        nop.engine = ins.engine
                        nop.sync_info = mybir.SyncInfo(on_wait=[w], on_update=[])
                        nc.register_instruction(nop, overwrite=True)
                        il.insert(i, nop)
                        i += 1
                i += 1


def _new_nc():
    return bass.Bass("TRN2", target_bir_lowering=False, debug=False,
                     enable_asserts=True, num_devices=8)


def _run(nc, in_maps, name):
    trace = os.environ.get("MAMBA_TRACE", "0") == "1"
    res = run_bass_kernel_spmd(nc, in_maps, core_ids=list(range(8)), trace=trace)
    if trace:
        EXEC_TIMES[name] = res.exec_time_ns
        TRACES[name] = res.instructions_and_trace
    return res.results


# ------------------------------------------------------------------- L1
def build_l1():
    nc = _new_nc()
    x_in = nc.dram_tensor("x_in", [C1, HH + 2, W], bf16, kind="ExternalInput")
    wcv1 = nc.dram_tensor("wcv1", [C1, Cm], bf16, kind="ExternalInput")      # lhsT
    bcv1 = nc.dram_tensor("bcv1", [Cm, 1], f32, kind="ExternalInput")
    wfold = nc.dram_tensor("wfold", [Cm, 9, Cm], bf16, kind="ExternalInput")  # (k, tap, m)
    bconv = nc.dram_tensor("bconv", [Cm, 1], f32, kind="ExternalInput")
    wz = nc.dram_tensor("wz", [Cm, Cm], bf16, kind="ExternalInput")          # lhsT
    hmask = nc.dram_tensor("hmask", [Cm, 2], f32, kind="ExternalInput")
    xc_out = nc.dram_tensor("xc_out", [Cm, LH], bf16, kind="ExternalOutput")
    z_out = nc.dram_tensor("z_out", [Cm, LH], bf16, kind="ExternalOutput")

    HP = HH + 2   # 66
    WP = W + 2    # 130

    with tile.TileContext(nc) as tc, \
         tc.tile_pool(name="w", bufs=1) as wp, \
         tc.tile_pool(name="d", bufs=1) as dp, \
         tc.tile_pool(name="ps", bufs=2, space="PSUM") as pp:
        tw1a = wp.tile([128, Cm], bf16)
        tw1b = wp.tile([128, Cm], bf16)
        nc.sync.dma_start(out=tw1a, in_=wcv1[0:128, :])
        nc.scalar.dma_start(out=tw1b, in_=wcv1[128:256, :])
        tb1 = wp.tile([Cm, 1], f32)
        nc.scalar.dma_start(out=tb1, in_=bcv1[:, :])
        tbc = wp.tile([Cm, 1], f32)
        nc.scalar.dma_start(out=tbc, in_=bconv[:, :])
        tmask = wp.tile([Cm, 2], f32)
        nc.scalar.dma_start(out=tmask, in_=hmask[:, :])

        # x as separate per-8-row-block tiles so each cv1 chunk gates only on
        # its own block's DMA, not the whole x load
        xblocks = [(rb, min(8, HP - rb)) for rb in range(0, HP, 8)]
        txa = [dp.tile([128, nr, W], bf16, name=f"txa{i}")
               for i, (_, nr) in enumerate(xblocks)]
        txb = [dp.tile([128, nr, W], bf16, name=f"txb{i}")
               for i, (_, nr) in enumerate(xblocks)]
        twf = wp.tile([Cm, 9, Cm], bf16)
        twz = wp.tile([Cm, Cm], bf16)
        for i, (rb, nr) in enumerate(xblocks):
            nc.sync.dma_start(out=txa[i], in_=x_in[0:128, rb:rb + nr, :])
            nc.scalar.dma_start(out=txb[i], in_=x_in[128:256, rb:rb + nr, :])
            if i == 1:  # fold/z weights after the first two x blocks
                nc.scalar.dma_start(out=twf, in_=wfold[:, :, :])
                nc.scalar.dma_start(out=twz, in_=wz[:, :])

        th = dp.tile([Cm, HP, WP], bf16)
        nc.vector.memset(th[:, :, 0:1], 0.0)
        nc.vector.memset(th[:, :, WP - 1:WP], 0.0)

        # cv1 over 66 rows: 16 chunks of 4 rows + 1 chunk of 2 rows.
        # Interleave cv1 with the fold chunks in PE issue order so the PE is
        # not queued behind the tail of the x load.
        row_chunks = [(r0, 4) for r0 in range(0, 64, 4)] + [(64, 2)]

        def cv1_chunk(idx):
            r0, nr = row_chunks[idx]
            blk, off = r0 // 8, r0 % 8
            pt = pp.tile([Cm, 512], f32, tag="cv1")
            nn = nr * W
            nc.tensor.matmul(out=pt[:, :nn], lhsT=tw1a[:, :],
                             rhs=txa[blk][:, off:off + nr, :], start=True, stop=False)
            nc.tensor.matmul(out=pt[:, :nn], lhsT=tw1b[:, :],
                             rhs=txb[blk][:, off:off + nr, :], start=False, stop=True)
            nc.vector.tensor_scalar_add(out=th[:, r0:r0 + nr, 1:W + 1],
                                        in0=pt[:, :nn], scalar1=tb1[:, 0:1])
            if idx == 0:
                nc.vector.tensor_scalar_mul(out=th[:, 0, :], in0=th[:, 0, :],
                                            scalar1=tmask[:, 0:1])
            if idx == len(row_chunks) - 1:
                nc.vector.tensor_scalar_mul(out=th[:, HP - 1, :],
                                            in0=th[:, HP - 1, :],
                                            scalar1=tmask[:, 1:2])

        cv1_chunk(0)
        cv1_chunk(1)

        txc = dp.tile([Cm, HH, W], bf16)
        tz = dp.tile([Cm, HH, W], bf16)
        for k in range(16):
            r0 = 4 * k
            pt = pp.tile([Cm, 512], f32, tag="fold")
            for t in range(9):
                dy, dx = t // 3 - 1, t % 3 - 1
                nc.tensor.matmul(
                    out=pt[:, :], lhsT=twf[:, t, :],
                    rhs=th[:, r0 + 1 + dy:r0 + 5 + dy, 1 + dx:W + 1 + dx],
                    start=(t == 0), stop=(t == 8))
            nc.scalar.activation(out=txc[:, r0:r0 + 4, :], in_=pt[:, :],
                                 func=AF.Silu, bias=tbc[:, :], scale=1.0)
            ptz = pp.tile([Cm, 512], f32, tag="z")
            nc.tensor.matmul(out=ptz[:, :], lhsT=twz[:, :],
                             rhs=th[:, r0 + 1:r0 + 5, 1:W + 1],
                             start=True, stop=True)
            nc.scalar.activation(out=tz[:, r0:r0 + 4, :], in_=ptz[:, :],
                                 func=AF.Silu, bias=0.0, scale=1.0)
            if k + 2 < len(row_chunks):
                cv1_chunk(k + 2)

            if r0 % 8 == 4:  # flush every 8 rows
                rs = r0 - 4
                nc.gpsimd.dma_start(out=xc_out[:, rs * W:(r0 + 4) * W],
                                    in_=txc[:, rs:r0 + 4, :])
                nc.gpsimd.dma_start(out=z_out[:, rs * W:(r0 + 4) * W],
                                  in_=tz[:, rs:r0 + 4, :])
    return nc


# ------------------------------------------------------------------- L2
def build_l2():
    nc = _new_nc()
    dts_f = nc.dram_tensor("dts_f", [R, L], bf16, kind="ExternalInput")
    dts_r = nc.dram_tensor("dts_r", [R, L], bf16, kind="ExternalInput")
    v_f = nc.dram_tensor("v_f", [Cm, L], bf16, kind="ExternalInput")
    v_r = nc.dram_tensor("v_r", [Cm, L], bf16, kind="ExternalInput")
    crow_f = nc.dram_tensor("crow_f", [1, L], bf16, kind="ExternalInput")
    crow_r = nc.dram_tensor("crow_r", [1, L], bf16, kind="ExternalInput")
    wdt_f = nc.dram_tensor("wdt_f", [R, Cm], bf16, kind="ExternalInput")  # lhsT
    wdt_r = nc.dram_tensor("wdt_r", [R, Cm], bf16, kind="ExternalInput")
    dtb_f = nc.dram_tensor("dtb_f", [Cm, 1], f32, kind="ExternalInput")
    dtb_r = nc.dram_tensor("dtb_r", [Cm, 1], f32, kind="ExternalInput")
    a_f = nc.dram_tensor("a_f", [Cm, 1], f32, kind="ExternalInput")
    a_r = nc.dram_tensor("a_r", [Cm, 1], f32, kind="ExternalInput")
    m_out = nc.dram_tensor("m_out", [Cm, L], bf16, kind="ExternalOutput")

    def bc_ap(t, sl):  # DRAM row slice -> partition-replicated AP
        return bass.AP(tensor=t, offset=sl.start, ap=[[0, 128], [1, sl.stop - sl.start]])

    with tile.TileContext(nc) as tc, \
         tc.tile_pool(name="w", bufs=1) as wp, \
         tc.tile_pool(name="full", bufs=1) as fp, \
         tc.tile_pool(name="ds", bufs=2) as dsp, \
         tc.tile_pool(name="vv", bufs=3) as vp, \
         tc.tile_pool(name="ck", bufs=3) as cp, \
         tc.tile_pool(name="hk", bufs=2) as hp, \
         tc.tile_pool(name="bc", bufs=3) as bp, \
         tc.tile_pool(name="mm", bufs=2) as mp, \
         tc.tile_pool(name="psd", bufs=2, space="PSUM") as psd:
        twf_ = wp.tile([R, Cm], bf16)
        twr_ = wp.tile([R, Cm], bf16)
        nc.sync.dma_start(out=twr_, in_=wdt_r[:, :])
        nc.sync.dma_start(out=twf_, in_=wdt_f[:, :])
        tbf = wp.tile([Cm, 1], f32)
        tbr = wp.tile([Cm, 1], f32)
        taf = wp.tile([Cm, 1], f32)
        tar = wp.tile([Cm, 1], f32)
        nc.scalar.dma_start(out=tbf, in_=dtb_f[:, :])
        nc.scalar.dma_start(out=tbr, in_=dtb_r[:, :])
        nc.scalar.dma_start(out=taf, in_=a_f[:, :])
        nc.scalar.dma_start(out=tar, in_=a_r[:, :])

        tmcf = fp.tile([Cm, L], bf16)   # h_f * C_f, natural position order
        tmcr = fp.tile([Cm, L], bf16)   # h_r * C_r, natural position order

        state = {"prev_f": None, "prev_r": None}

        def side(ci, rev, split=False):
            sl = slice(ci * CH, (ci + 1) * CH)
            sfx = "r" if rev else "f"
            tdts, tv_d, tcr, tw, tb, ta = (
                (dts_r, v_r, crow_r, twr_, tbr, tar) if rev else
                (dts_f, v_f, crow_f, twf_, tbf, taf))
            td = dsp.tile([R, CH], bf16, tag="d" + sfx)
            nc.gpsimd.dma_start(out=td, in_=tdts[:, sl])
            tv = vp.tile([Cm, CH], bf16, tag="v" + sfx)
            nc.sync.dma_start(out=tv, in_=tv_d[:, sl])
            tcb = bp.tile([Cm, CH], bf16, tag="c" + sfx)
            nc.gpsimd.dma_start(out=tcb, in_=bc_ap(tcr, sl))

            pt = psd.tile([Cm, CH], f32, tag="dtd")
            for j in range(CH // 512):
                nc.tensor.matmul(
                    out=pt[:, j * 512:(j + 1) * 512], lhsT=tw[:, :],
                    rhs=td[:, j * 512:(j + 1) * 512], start=True, stop=True)
            te1 = cp.tile([Cm, CH], bf16, tag="e1")
            tdt = cp.tile([Cm, CH], bf16, tag="dt")
            tav = cp.tile([Cm, CH], bf16, tag="av")
            tbt = cp.tile([Cm, CH], bf16, tag="bt")
            thc = hp.tile([Cm, CH], bf16, tag="h" + sfx)
            # For the very first chunk, run the chain in halves so the first
            # scan starts ~3us earlier (reverse dir: high half scans first).
            halves = ([slice(CH // 2, CH), slice(0, CH // 2)] if split
                      else [slice(0, CH)])
            for hs in halves:
                nc.scalar.activation(out=te1[:, hs], in_=pt[:, hs], func=AF.Exp,
                                     bias=tb[:, :], scale=1.0)
                nc.scalar.activation(out=tdt[:, hs], in_=te1[:, hs], func=AF.Ln,
                                     bias=1.0, scale=1.0)
                nc.scalar.activation(out=tav[:, hs], in_=tdt[:, hs], func=AF.Exp,
                                     bias=0.0, scale=ta[:, :])
                nc.vector.tensor_tensor(out=tbt[:, hs], in0=tdt[:, hs],
                                        in1=tv[:, hs], op=MULT)
                if rev:
                    prev = state["prev_r"]
                    nc.vector.tensor_tensor_scan(
                        out=thc[:, hs][:, ::-1], data0=tav[:, hs][:, ::-1],
                        data1=tbt[:, hs][:, ::-1],
                        initial=0.0 if prev is None else prev,
                        op0=MULT, op1=ADD)
                    state["prev_r"] = thc[:, hs.start:hs.start + 1]
                else:
                    prev = state["prev_f"]
                    nc.vector.tensor_tensor_scan(
                        out=thc[:, hs], data0=tav[:, hs], data1=tbt[:, hs],
                        initial=0.0 if prev is None else prev,
                        op0=MULT, op1=ADD)
                    state["prev_f"] = thc[:, hs.stop - 1:hs.stop]
            if rev:
                nc.vector.tensor_tensor(out=tmcr[:, sl], in0=thc, in1=tcb, op=MULT)
            else:
                nc.vector.tensor_tensor(out=tmcf[:, sl], in0=thc, in1=tcb, op=MULT)

        for s in range(NCH):
            side(NCH - 1 - s, rev=True, split=(s == 0))
            side(s, rev=False)
            if s >= NCH // 2:
                for c in (s, NCH - 1 - s):
                    slc = slice(c * CH, (c + 1) * CH)
                    tm = mp.tile([Cm, CH], bf16, tag="m")
                    nc.vector.tensor_tensor(out=tm, in0=tmcf[:, slc],
                                            in1=tmcr[:, slc], op=ADD)
                    nc.scalar.dma_start(out=m_out[:, slc], in_=tm)
    return nc


# ------------------------------------------------------------------- L3
def build_l3():
    nc = _new_nc()
    m02 = nc.dram_tensor("m02", [Cm, LH], bf16, kind="ExternalInput")
    m13 = nc.dram_tensor("m13", [Cm, LH], bf16, kind="ExternalInput")
    z_in = nc.dram_tensor("z_in", [Cm, LH], bf16, kind="ExternalInput")
    xc_in = nc.dram_tensor("xc_in", [Cm, LH], bf16, kind="ExternalInput")
    dtot = nc.dram_tensor("dtot", [Cm, 1], f32, kind="ExternalInput")
    wfin = nc.dram_tensor("wfin", [Cm, C2], bf16, kind="ExternalInput")
    ones128 = nc.dram_tensor("ones128", [Cm, 1], f32, kind="ExternalInput")
    rrow = nc.dram_tensor("rrow", [1, LH], bf16, kind="Internal")
    mrow = nc.dram_tensor("mrow", [1, LH], bf16, kind="Internal")
    d_out = nc.dram_tensor("d_out", [C2, LH], bf16, kind="ExternalOutput")

    QL = LH // 128  # 64

    def bc_ap(t, sl):  # DRAM row slice -> partition-replicated AP
        return bass.AP(tensor=t, offset=sl.start, ap=[[0, 128], [1, sl.stop - sl.start]])

    with tile.TileContext(nc) as tc, \
         tc.tile_pool(name="w", bufs=1) as wp, \
         tc.tile_pool(name="d", bufs=1) as dp, \
         tc.tile_pool(name="c", bufs=2) as cp, \
         tc.tile_pool(name="rb", bufs=2) as bp, \
         tc.tile_pool(name="st", bufs=1) as sp, \
         tc.tile_pool(name="ps1", bufs=2, space="PSUM") as ps1, \
         tc.tile_pool(name="ps3", bufs=2, space="PSUM") as ps3:
        twa = wp.tile([Cm, 128], bf16)
        twb = wp.tile([Cm, 128], bf16)
        nc.sync.dma_start(out=twa, in_=wfin[:, 0:128])
        nc.sync.dma_start(out=twb, in_=wfin[:, 128:256])
        tone = wp.tile([Cm, 1], bf16)
        nc.gpsimd.dma_start(out=tone, in_=ones128[:, :])
        tdt = wp.tile([Cm, 1], f32)
        nc.scalar.dma_start(out=tdt, in_=dtot[:, :])
        teps = wp.tile([128, 1], f32)
        nc.vector.memset(teps, 1e-5)

        tm0 = dp.tile([Cm, LH], bf16)
        tm1 = dp.tile([Cm, LH], bf16)
        tz = dp.tile([Cm, LH], bf16)
        txc = dp.tile([Cm, LH], bf16)
        for ci in range(NC3):
            s = slice(ci * CH3, (ci + 1) * CH3)
            nc.sync.dma_start(out=tm0[:, s], in_=m02[:, s])
            nc.scalar.dma_start(out=tm1[:, s], in_=m13[:, s])
            nc.sync.dma_start(out=txc[:, s], in_=xc_in[:, s])
            nc.scalar.dma_start(out=tz[:, s], in_=z_in[:, s])

        ty = dp.tile([Cm, LH], bf16)        # y = m0 + m1 + Dtot*xc
        QC = CH3 // 128                     # 16 row-gather cols per chunk

        def pass1_rows(ci):
            sl = slice(ci * CH3, (ci + 1) * CH3)
            tdx = cp.tile([Cm, CH3], bf16, tag="dx")
            nc.vector.tensor_scalar_mul(out=tdx, in0=txc[:, sl],
                                        scalar1=tdt[:, 0:1])
            ta_ = cp.tile([Cm, CH3], bf16, tag="tya")
            nc.vector.tensor_tensor(out=ta_, in0=tm0[:, sl], in1=tm1[:, sl], op=ADD)
            nc.vector.tensor_tensor(out=ty[:, sl], in0=ta_, in1=tdx, op=ADD)
            tsq = cp.tile([Cm, CH3], bf16, tag="sq")
            nc.vector.tensor_tensor(out=tsq, in0=ty[:, sl], in1=ty[:, sl], op=MULT)
            trowc = cp.tile([33, CH3], bf16, tag="rw")
            for j in range(CH3 // 512):
                s2 = slice(j * 512, (j + 1) * 512)
                s2g = slice(ci * CH3 + j * 512, ci * CH3 + (j + 1) * 512)
                pst = ps1.tile([33, 512], f32, tag="st")
                nc.tensor.matmul(out=pst[0:1, :], lhsT=tone[:, :],
                                 rhs=ty[:, s2g], start=True, stop=True)
                nc.tensor.matmul(out=pst[32:33, :], lhsT=tone[:, :],
                                 rhs=tsq[:, s2], start=True, stop=True)
                nc.scalar.activation(out=trowc[:, s2], in_=pst[:, :],
                                     func=AF.Identity, bias=0.0, scale=1.0)

            tmu2 = cp.tile([128, QC], bf16, tag="r1")
            tss2 = cp.tile([128, QC], bf16, tag="r2")
            nc.sync.dma_start(out=tmu2[:, :], in_=trowc[0:1, :])
            nc.scalar.dma_start(out=tss2[:, :], in_=trowc[32:33, :])
            tvar = cp.tile([128, QC], f32, tag="r3")
            nc.vector.tensor_tensor(out=tvar, in0=tmu2, in1=tmu2, op=MULT)
            nc.vector.tensor_tensor(out=tvar, in0=tss2, in1=tvar, op=SUB)
            tlnv = cp.tile([128, QC], f32, tag="r4")
            nc.scalar.activation(out=tlnv, in_=tvar, func=AF.Ln,
                                 bias=teps[:, :], scale=1.0)
            trst = cp.tile([128, QC], bf16, tag="r5")
            nc.scalar.activation(out=trst, in_=tlnv, func=AF.Exp,
                                 bias=0.0, scale=-0.5)
            tmr = cp.tile([128, QC], bf16, tag="r6")
            nc.vector.tensor_tensor(out=tmr, in0=tmu2, in1=trst, op=MULT)
            nc.vector.tensor_scalar_mul(out=tmr, in0=tmr, scalar1=-1.0)
            nc.sync.dma_start(out=rrow[0:1, sl], in_=trst[:, :])
            nc.scalar.dma_start(out=mrow[0:1, sl], in_=tmr[:, :])

        def pass2(ci):
            # normalize + z-gate + final matmul for this chunk
            sl = slice(ci * CH3, (ci + 1) * CH3)
            tRb = bp.tile([Cm, CH3], bf16, tag="Rb")
            nc.scalar.dma_start(out=tRb, in_=bc_ap(rrow, sl))
            tSb = bp.tile([Cm, CH3], bf16, tag="Sb")
            nc.sync.dma_start(out=tSb, in_=bc_ap(mrow, sl))
            tt1 = cp.tile([Cm, CH3], bf16, tag="t1")
            nc.vector.tensor_tensor(out=tt1, in0=ty[:, sl], in1=tRb, op=MULT)
            tt2 = cp.tile([Cm, CH3], bf16, tag="t2")
            nc.vector.tensor_tensor(out=tt2, in0=tt1, in1=tSb, op=ADD)
            tt3 = cp.tile([Cm, CH3], bf16, tag="t3")
            nc.vector.tensor_tensor(out=tt3, in0=tt2, in1=tz[:, sl], op=MULT)
            for j in range(CH3 // 512):
                s2 = slice(j * 512, (j + 1) * 512)
                s2g = slice(ci * CH3 + j * 512, ci * CH3 + (j + 1) * 512)
                pd = ps3.tile([128, 1024], f32, tag="d")
                nc.tensor.matmul(out=pd[:, 0:512], lhsT=twa[:, :],
                                 rhs=tt3[:, s2], start=True, stop=True)
                nc.tensor.matmul(out=pd[:, 512:1024], lhsT=twb[:, :],
                                 rhs=tt3[:, s2], start=True, stop=True)
                td_ = cp.tile([128, 1024], bf16, tag="td")
                if (ci * 4 + j) % 3 == 2:
                    nc.vector.tensor_copy(out=td_, in_=pd[:, :])
                else:
                    nc.scalar.activation(out=td_, in_=pd[:, :],
                                         func=AF.Identity, bias=0.0, scale=1.0)
                nc.gpsimd.dma_start(out=d_out[0:128, s2g], in_=td_[:, 0:512])
                nc.sync.dma_start(out=d_out[128:256, s2g], in_=td_[:, 512:1024])

        # software pipeline: pass2 lags pass1 by one chunk so the engines never
        # sit on the bcast-row round trip
        for ci in range(NC3):
            pass1_rows(ci)
            if ci >= 1:
                pass2(ci - 1)
        pass2(NC3 - 1)
    return nc


# ------------------------------------------------------------------- host
def _get_ncs():
    if "ncs" not in _CACHE:
        nc1, nc2, nc3 = build_l1(), build_l2(), build_l3()
        for n in (nc1, nc2, nc3):
            _split_multiwaits(n)
        _CACHE["ncs"] = (nc1, nc2, nc3)
    return _CACHE["ncs"]


def kernel(x, cv1_w, cv1_b, scale_w, in_proj_w, conv_w, conv_b, x_proj_w,
           dt_w, dt_b, A_logs, Ds, ln_g, ln_b, out_proj_w, cv2_w, cv2_b):
    f = np.float32
    x = np.asarray(x, f)
    cv1_w = np.asarray(cv1_w, f); cv1_b = np.asarray(cv1_b, f)
    in_proj_w = np.asarray(in_proj_w, f)
    conv_w = np.asarray(conv_w, f); conv_b = np.asarray(conv_b, f)
    x_proj_w = np.asarray(x_proj_w, f)
    dt_w = np.asarray(dt_w, f); dt_b = np.asarray(dt_b, f)
    A_logs = np.asarray(A_logs, f); Ds = np.asarray(Ds, f)
    ln_g = np.asarray(ln_g, f); ln_b = np.asarray(ln_b, f)
    out_proj_w = np.asarray(out_proj_w, f)
    cv2_w = np.asarray(cv2_w, f); cv2_b = np.asarray(cv2_b, f)
    scale_v = np.asarray(scale_w, f).reshape(Cm)

    Wip_x, Wip_z = in_proj_w[:Cm], in_proj_w[Cm:]
    dwk = conv_w[:, 0]
    A = -np.exp(A_logs).reshape(K, Cm)
    Dk = Ds.reshape(K, Cm)
    Dtot = Dk.sum(axis=0)                              # (Cm,)
    Wdts8 = x_proj_w[:, :R]                            # (K, R, Cm)
    WB, WC = x_proj_w[:, R], x_proj_w[:, R + 1]        # (K, Cm)
    W_final = cv2_w @ (scale_v[:, None] * out_proj_w)
    W_final_g = W_final * ln_g[None, :]

    # fold lhsT: (tap, k=h-chan, m=out-chan) -> host layout (k, tap, m)
    Wfold = np.einsum('cyx,cd->yxdc', dwk, Wip_x)      # (3,3, in, out)
    wfold_rm = np.ascontiguousarray(
        Wfold.reshape(9, Cm, Cm).transpose(1, 0, 2))   # row-major cores

    nc1, nc2, nc3 = _get_ncs()

    # ---------------- L1 ----------------
    l1_maps = []
    for core in range(8):
        b, half = core // 2, core % 2
        r0 = half * HH
        xs = np.zeros((C1, HH + 2, W), NBF)
        lo, hi = r0 - 1, r0 + HH + 1
        slo, shi = max(lo, 0), min(hi, H)
        xs[:, slo - lo: shi - lo, :] = x[b, :, slo:shi, :].astype(NBF)
        mask = np.ones((Cm, 2), np.float32)
        mask[:, 0] = 0.0 if half == 0 else 1.0
        mask[:, 1] = 1.0 if half == 0 else 0.0
        l1_maps.append({
            "x_in": xs,
            "wcv1": np.ascontiguousarray(cv1_w.T).astype(NBF),
            "bcv1": cv1_b.reshape(Cm, 1),
            "wfold": wfold_rm.astype(NBF),
            "bconv": conv_b.reshape(Cm, 1),
            "wz": np.ascontiguousarray(Wip_z.T).astype(NBF),
            "hmask": mask,
        })
    r1 = _run(nc1, l1_maps, "L1")

    xc = np.zeros((B, Cm, L), NBF)
    zf = np.zeros((B, Cm, L), NBF)
    for core in range(8):
        b, half = core // 2, core % 2
        sl = slice(half * LH, (half + 1) * LH)
        xc[b][:, sl] = r1[core]["xc_out"]
        zf[b][:, sl] = r1[core]["z_out"]

    # ---------------- L2 ----------------
    def t_spatial(a):
        return np.ascontiguousarray(
            a.reshape(*a.shape[:-1], H, W).swapaxes(-1, -2).reshape(*a.shape[:-1], L))

    l2_maps = []
    for core in range(8):
        b, g = core // 2, core % 2
        if g == 0:
            u = xc[b]
            kf, kr = 0, 2
        else:
            u = t_spatial(xc[b])
            kf, kr = 1, 3
        uf = u.astype(np.float32)
        m = {}
        for sfx, k in (("f", kf), ("r", kr)):
            m["dts_" + sfx] = (Wdts8[k] @ uf).astype(NBF)
            brow = WB[k] @ uf
            m["crow_" + sfx] = (WC[k] @ uf).astype(NBF).reshape(1, L)
            m["v_" + sfx] = (uf * brow[None, :]).astype(NBF)
            m["wdt_" + sfx] = np.ascontiguousarray(dt_w[k].T).astype(NBF)  # (R, Cm)
            m["dtb_" + sfx] = dt_b[k].reshape(Cm, 1)
            m["a_" + sfx] = A[k].reshape(Cm, 1).astype(f)
        l2_maps.append(m)
    r2 = _run(nc2, l2_maps, "L2")

    # ---------------- L3 ----------------
    l3_maps = []
    for b in range(B):
        m02 = r2[2 * b]["m_out"]
        m13t = t_spatial(r2[2 * b + 1]["m_out"])
        for half in range(2):
            sl = slice(half * LH, (half + 1) * LH)
            l3_maps.append({
                "m02": np.ascontiguousarray(m02[:, sl]),
                "m13": np.ascontiguousarray(m13t[:, sl]),
                "z_in": np.ascontiguousarray(zf[b][:, sl]),
                "xc_in": np.ascontiguousarray(xc[b][:, sl]),
                "dtot": Dtot.reshape(Cm, 1),
                "wfin": np.ascontiguousarray(W_final_g.T).astype(NBF),
                "ones128": np.full((Cm, 1), 1.0 / Cm, np.float32),
            })
    r3 = _run(nc3, l3_maps, "L3")

    out = np.empty((B, C2, H, W), np.float32)
    for core in range(8):
        b, half = core // 2, core % 2
        sl = slice(half * LH, (half + 1) * LH)
        out[b].reshape(C2, L)[:, sl] = r3[core]["d_out"]
    if np.any(ln_b != 0.0):
        # ln beta term, folded past the z-gate on the host: W_final @ (b .* z)
        Wb = W_final * ln_b[None, :]
        for b in range(B):
            out[b].reshape(C2, L)[:, :] += (Wb @ zf[b].astype(np.float32))
    out += x
    out += cv2_b[None, :, None, None]
    return out


# revision 27
# speedup vs baseline: 1.2897x; 1.2897x over previous
"""BottleneckMamba Trainium2 kernel (self-contained), v2.

out = x + cv2( scale * out_proj( LN(cross-merge(4-dir selective scan(N=1))) * z ) )

3 SPMD launches on 8 NeuronCores:
  L1 (core=(b, image-half)): cv1 -> h (bias on DVE); depthwise3x3*in_proj
     folded into 9 matmuls -> silu -> xc ; z = silu(Wz@h).
  L2 (core=(b, dir-group)): per direction: dtd = dt_w8 @ dts (rank-8 rows
     preshipped) -> exp/ln1p/exp on ACT (one table set) -> tbt = dt*v
     (v = u*B preshipped) -> tensor_tensor_scan -> mc = h*C (C broadcast by
     replicating DMA). Chunks processed ping-pong (reverse dir from the top,
     forward from the bottom) so both scans stream concurrently; m = mcf+mcr.
  L3 (core=(b, half)): y = m02 + m13^T + (sum_k D_k).xc; LayerNorm stats via
     ones-matmul per position chunk (no global barrier: stats are
     per-position), rstd/-mu*rstd rows written to DRAM and replicated back by
     DMA, normalize * z, fused (cv2 @ diag(scale) @ out_proj @ diag(ln_g))
     matmul -> bf16 delta.
Host: shards/reassembles, transposes between launches, computes the rank-8
dt rows + B/C rows + v = u*B from xc (bf16 data prep), adds residual x,
cv2 bias and the (zero here) ln_b term.
"""
import os
import sys

sys.path.insert(0, '/opt/trn_rl_repo')

import numpy as np
import ml_dtypes

import concourse.bass as bass
import concourse.tile as tile
import concourse.mybir as mybir
from concourse.bass_utils import run_bass_kernel_spmd

bf16 = mybir.dt.bfloat16
f32 = mybir.dt.float32
MULT, ADD = mybir.AluOpType.mult, mybir.AluOpType.add
SUB = mybir.AluOpType.subtract
AF = mybir.ActivationFunctionType
NBF = ml_dtypes.bfloat16

B, C1, C2, H, W = 4, 256, 256, 128, 128
Cm, K, R = 128, 4, 8
L = H * W          # 16384
HH = H // 2        # 64 rows per half
LH = HH * W        # 8192
CH = 2048          # L2 chunk
NCH = L // CH      # 8
CH3 = 2048         # L3 chunk
NC3 = LH // CH3    # 4

EXEC_TIMES = {}    # launch -> exec ns (MAMBA_TRACE=1)
TRACES = {}        # launch -> (insts, trace_path) (MAMBA_TRACE=1)
_CACHE = {}


def _split_multiwaits(nc):
    """walrus here accepts ONE sync-wait per instruction; hoist extras into
    single-wait same-engine NOPs inserted before the instruction."""
    for f in nc.m.functions:
        for bb in f.blocks:
            il = bb.instructions
            i = 0
            while i < len(il):
                ins = il[i]
                si = getattr(ins, "sync_info", None)
                if si is not None and len(si.on_wait) > 1:
                    waits = list(si.on_wait)
                    ins.sync_info = mybir.SyncInfo(
                        on_wait=[waits[-1]], on_update=list(si.on_update))
                    for w in waits[:-1]:
                        nop = mybir.InstNoOp(
                            name=nc.get_next_instruction_name(), ins=[], outs=[])
                        nop.engine = ins.engine
                        nop.sync_info = mybir.SyncInfo(on_wait=[w], on_update=[])
                        nc.register_instruction(nop, overwrite=True)
                        il.insert(i, nop)
                        i += 1
                i += 1


def _new_nc():
    return bass.Bass("TRN2", target_bir_lowering=False, debug=False,
                     enable_asserts=True, num_devices=8)


def _run(nc, in_maps, name):
    trace = os.environ.get("MAMBA_TRACE", "0") == "1"
    res = run_bass_kernel_spmd(nc, in_maps, core_ids=list(range(8)), trace=trace)
    if trace:
        EXEC_TIMES[name] = res.exec_time_ns
        TRACES[name] = res.instructions_and_trace
    return res.results


# ------------------------------------------------------------------- L1
def build_l1():
    nc = _new_nc()
    x_in = nc.dram_tensor("x_in", [C1, HH + 2, W], bf16, kind="ExternalInput")
    wcv1 = nc.dram_tensor("wcv1", [C1, Cm], bf16, kind="ExternalInput")      # lhsT
    bcv1 = nc.dram_tensor("bcv1", [Cm, 1], f32, kind="ExternalInput")
    wfold = nc.dram_tensor("wfold", [Cm, 9, Cm], bf16, kind="ExternalInput")  # (k, tap, m)
    bconv = nc.dram_tensor("bconv", [Cm, 1], f32, kind="ExternalInput")
    wz = nc.dram_tensor("wz", [Cm, Cm], bf16, kind="ExternalInput")          # lhsT
    hmask = nc.dram_tensor("hmask", [Cm, 2], f32, kind="ExternalInput")
    xc_out = nc.dram_tensor("xc_out", [Cm, LH], bf16, kind="ExternalOutput")
    z_out = nc.dram_tensor("z_out", [Cm, LH], bf16, kind="ExternalOutput")

    HP = HH + 2   # 66
    WP = W + 2    # 130

    with tile.TileContext(nc) as tc, \
         tc.tile_pool(name="w", bufs=1) as wp, \
         tc.tile_pool(name="d", bufs=1) as dp, \
         tc.tile_pool(name="ps", bufs=2, space="PSUM") as pp:
        tw1a = wp.tile([128, Cm], bf16)
        tw1b = wp.tile([128, Cm], bf16)
        nc.sync.dma_start(out=tw1a, in_=wcv1[0:128, :])
        nc.scalar.dma_start(out=tw1b, in_=wcv1[128:256, :])
        tb1 = wp.tile([Cm, 1], f32)
        nc.scalar.dma_start(out=tb1, in_=bcv1[:, :])
        tbc = wp.tile([Cm, 1], f32)
        nc.scalar.dma_start(out=tbc, in_=bconv[:, :])
        tmask = wp.tile([Cm, 2], f32)
        nc.scalar.dma_start(out=tmask, in_=hmask[:, :])

        # x as separate per-8-row-block tiles so each cv1 chunk gates only on
        # its own block's DMA, not the whole x load
        xblocks = [(rb, min(8, HP - rb)) for rb in range(0, HP, 8)]
        txa = [dp.tile([128, nr, W], bf16, name=f"txa{i}")
               for i, (_, nr) in enumerate(xblocks)]
        txb = [dp.tile([128, nr, W], bf16, name=f"txb{i}")
               for i, (_, nr) in enumerate(xblocks)]
        twf = wp.tile([Cm, 9, Cm], bf16)
        twz = wp.tile([Cm, Cm], bf16)
        for i, (rb, nr) in enumerate(xblocks):
            nc.sync.dma_start(out=txa[i], in_=x_in[0:128, rb:rb + nr, :])
            nc.scalar.dma_start(out=txb[i], in_=x_in[128:256, rb:rb + nr, :])
            if i == 1:  # fold/z weights after the first two x blocks
                nc.scalar.dma_start(out=twf, in_=wfold[:, :, :])
                nc.scalar.dma_start(out=twz, in_=wz[:, :])

        th = dp.tile([Cm, HP, WP], bf16)
        nc.vector.memset(th[:, :, 0:1], 0.0)
        nc.vector.memset(th[:, :, WP - 1:WP], 0.0)

        # cv1 over 66 rows: 16 chunks of 4 rows + 1 chunk of 2 rows.
        # Interleave cv1 with the fold chunks in PE issue order so the PE is
        # not queued behind the tail of the x load.
        row_chunks = [(r0, 4) for r0 in range(0, 64, 4)] + [(64, 2)]

        def cv1_chunk(idx):
            r0, nr = row_chunks[idx]
            blk, off = r0 // 8, r0 % 8
            pt = pp.tile([Cm, 512], f32, tag="cv1")
            nn = nr * W
            nc.tensor.matmul(out=pt[:, :nn], lhsT=tw1a[:, :],
                             rhs=txa[blk][:, off:off + nr, :], start=True, stop=False)
            nc.tensor.matmul(out=pt[:, :nn], lhsT=tw1b[:, :],
                             rhs=txb[blk][:, off:off + nr, :], start=False, stop=True)
            nc.vector.tensor_scalar_add(out=th[:, r0:r0 + nr, 1:W + 1],
                                        in0=pt[:, :nn], scalar1=tb1[:, 0:1])
            if idx == 0:
                nc.vector.tensor_scalar_mul(out=th[:, 0, :], in0=th[:, 0, :],
                                            scalar1=tmask[:, 0:1])
            if idx == len(row_chunks) - 1:
                nc.vector.tensor_scalar_mul(out=th[:, HP - 1, :],
                                            in0=th[:, HP - 1, :],
                                            scalar1=tmask[:, 1:2])

        cv1_chunk(0)
        cv1_chunk(1)

        txc = dp.tile([Cm, HH, W], bf16)
        tz = dp.tile([Cm, HH, W], bf16)
        for k in range(16):
            r0 = 4 * k
            pt = pp.tile([Cm, 512], f32, tag="fold")
            for t in range(9):
                dy, dx = t // 3 - 1, t % 3 - 1
                nc.tensor.matmul(
                    out=pt[:, :], lhsT=twf[:, t, :],
                    rhs=th[:, r0 + 1 + dy:r0 + 5 + dy, 1 + dx:W + 1 + dx],
                    start=(t == 0), stop=(t == 8))
            nc.scalar.activation(out=txc[:, r0:r0 + 4, :], in_=pt[:, :],
                                 func=AF.Silu, bias=tbc[:, :], scale=1.0)
            ptz = pp.tile([Cm, 512], f32, tag="z")
            nc.tensor.matmul(out=ptz[:, :], lhsT=twz[:, :],
                             rhs=th[:, r0 + 1:r0 + 5, 1:W + 1],
                             start=True, stop=True)
            nc.scalar.activation(out=tz[:, r0:r0 + 4, :], in_=ptz[:, :],
                                 func=AF.Silu, bias=0.0, scale=1.0)
            if k + 2 < len(row_chunks):
                cv1_chunk(k + 2)

            if r0 % 8 == 4:  # flush every 8 rows
                rs = r0 - 4
                nc.gpsimd.dma_start(out=xc_out[:, rs * W:(r0 + 4) * W],
                                    in_=txc[:, rs:r0 + 4, :])
                nc.gpsimd.dma_start(out=z_out[:, rs * W:(r0 + 4) * W],
                                  in_=tz[:, rs:r0 + 4, :])
    return nc


# ------------------------------------------------------------------- L2
def build_l2():
    nc = _new_nc()
    dts_f = nc.dram_tensor("dts_f", [R, L], bf16, kind="ExternalInput")
    dts_r = nc.dram_tensor("dts_r", [R, L], bf16, kind="ExternalInput")
    v_f = nc.dram_tensor("v_f", [Cm, L], bf16, kind="ExternalInput")
    v_r = nc.dram_tensor("v_r", [Cm, L], bf16, kind="ExternalInput")
    crow_f = nc.dram_tensor("crow_f", [1, L], bf16, kind="ExternalInput")
    crow_r = nc.dram_tensor("crow_r", [1, L], bf16, kind="ExternalInput")
    wdt_f = nc.dram_tensor("wdt_f", [R, Cm], bf16, kind="ExternalInput")  # lhsT
    wdt_r = nc.dram_tensor("wdt_r", [R, Cm], bf16, kind="ExternalInput")
    dtb_f = nc.dram_tensor("dtb_f", [Cm, 1], f32, kind="ExternalInput")
    dtb_r = nc.dram_tensor("dtb_r", [Cm, 1], f32, kind="ExternalInput")
    a_f = nc.dram_tensor("a_f", [Cm, 1], f32, kind="ExternalInput")
    a_r = nc.dram_tensor("a_r", [Cm, 1], f32, kind="ExternalInput")
    m_out = nc.dram_tensor("m_out", [Cm, L], bf16, kind="ExternalOutput")

    def bc_ap(t, sl):  # DRAM row slice -> partition-replicated AP
        return bass.AP(tensor=t, offset=sl.start, ap=[[0, 128], [1, sl.stop - sl.start]])

    with tile.TileContext(nc) as tc, \
         tc.tile_pool(name="w", bufs=1) as wp, \
         tc.tile_pool(name="full", bufs=1) as fp, \
         tc.tile_pool(name="ds", bufs=2) as dsp, \
         tc.tile_pool(name="vv", bufs=3) as vp, \
         tc.tile_pool(name="ck", bufs=2) as cp, \
         tc.tile_pool(name="hk", bufs=2) as hp, \
         tc.tile_pool(name="bc", bufs=3) as bp, \
         tc.tile_pool(name="mm", bufs=2) as mp, \
         tc.tile_pool(name="psd", bufs=2, space="PSUM") as psd:
        twf_ = wp.tile([R, Cm], bf16)
        twr_ = wp.tile([R, Cm], bf16)
        nc.sync.dma_start(out=twr_, in_=wdt_r[:, :])
        nc.sync.dma_start(out=twf_, in_=wdt_f[:, :])
        tbf = wp.tile([Cm, 1], f32)
        tbr = wp.tile([Cm, 1], f32)
        taf = wp.tile([Cm, 1], f32)
        tar = wp.tile([Cm, 1], f32)
        nc.scalar.dma_start(out=tbf, in_=dtb_f[:, :])
        nc.scalar.dma_start(out=tbr, in_=dtb_r[:, :])
        nc.scalar.dma_start(out=taf, in_=a_f[:, :])
        nc.scalar.dma_start(out=tar, in_=a_r[:, :])

        tmcf = fp.tile([Cm, L], bf16)   # h_f * C_f, natural position order
        tmcr = fp.tile([Cm, L], bf16)   # h_r * C_r, natural position order

        state = {"prev_f": None, "prev_r": None}

        def side(ci, rev, split=False):
            sl = slice(ci * CH, (ci + 1) * CH)
            sfx = "r" if rev else "f"
            tdts, tv_d, tcr, tw, tb, ta = (
                (dts_r, v_r, crow_r, twr_, tbr, tar) if rev else
                (dts_f, v_f, crow_f, twf_, tbf, taf))
            td = dsp.tile([R, CH], bf16, tag="d" + sfx)
            nc.gpsimd.dma_start(out=td, in_=tdts[:, sl])
            tv = vp.tile([Cm, CH], bf16, tag="v" + sfx)
            nc.sync.dma_start(out=tv, in_=tv_d[:, sl])
            tcb = bp.tile([Cm, CH], bf16, tag="c" + sfx)
            nc.gpsimd.dma_start(out=tcb, in_=bc_ap(tcr, sl))

            pt = psd.tile([Cm, CH], f32, tag="dtd")
            for j in range(CH // 512):
                nc.tensor.matmul(
                    out=pt[:, j * 512:(j + 1) * 512], lhsT=tw[:, :],
                    rhs=td[:, j * 512:(j + 1) * 512], start=True, stop=True)
            te1 = cp.tile([Cm, CH], bf16, tag="e1")
            tdt = cp.tile([Cm, CH], bf16, tag="dt")
            tav = cp.tile([Cm, CH], bf16, tag="av")
            tbt = cp.tile([Cm, CH], bf16, tag="bt")
            thc = hp.tile([Cm, CH], bf16, tag="h" + sfx)
            # For the very first chunk, run the chain in halves so the first
            # scan starts ~3us earlier (reverse dir: high half scans first).
            halves = ([slice(CH // 2, CH), slice(0, CH // 2)] if split
                      else [slice(0, CH)])
            for hs in halves:
                nc.scalar.activation(out=te1[:, hs], in_=pt[:, hs], func=AF.Exp,
                                     bias=tb[:, :], scale=1.0)
                nc.scalar.activation(out=tdt[:, hs], in_=te1[:, hs], func=AF.Ln,
                                     bias=1.0, scale=1.0)
                nc.scalar.activation(out=tav[:, hs], in_=tdt[:, hs], func=AF.Exp,
                                     bias=0.0, scale=ta[:, :])
                nc.vector.tensor_tensor(out=tbt[:, hs], in0=tdt[:, hs],
                                        in1=tv[:, hs], op=MULT)
                if rev:
                    prev = state["prev_r"]
                    nc.vector.tensor_tensor_scan(
                        out=thc[:, hs][:, ::-1], data0=tav[:, hs][:, ::-1],
                        data1=tbt[:, hs][:, ::-1],
                        initial=0.0 if prev is None else prev,
                        op0=MULT, op1=ADD)
                    state["prev_r"] = thc[:, hs.start:hs.start + 1]
                else:
                    prev = state["prev_f"]
                    nc.vector.tensor_tensor_scan(
                        out=thc[:, hs], data0=tav[:, hs], data1=tbt[:, hs],
                        initial=0.0 if prev is None else prev,
                        op0=MULT, op1=ADD)
                    state["prev_f"] = thc[:, hs.stop - 1:hs.stop]
            if rev:
                nc.vector.tensor_tensor(out=tmcr[:, sl], in0=thc, in1=tcb, op=MULT)
            else:
                nc.vector.tensor_tensor(out=tmcf[:, sl], in0=thc, in1=tcb, op=MULT)

        for s in range(NCH):
            side(NCH - 1 - s, rev=True, split=(s == 0))
            side(s, rev=False)
            if s >= NCH // 2:
                for c in (s, NCH - 1 - s):
                    slc = slice(c * CH, (c + 1) * CH)
                    tm = mp.tile([Cm, CH], bf16, tag="m")
                    nc.vector.tensor_tensor(out=tm, in0=tmcf[:, slc],
                                            in1=tmcr[:, slc], op=ADD)
                    nc.scalar.dma_start(out=m_out[:, slc], in_=tm)
    return nc


# ------------------------------------------------------------------- L3
def build_l3():
    nc = _new_nc()
    m02 = nc.dram_tensor("m02", [Cm, LH], bf16, kind="ExternalInput")
    m13 = nc.dram_tensor("m13", [Cm, LH], bf16, kind="ExternalInput")
    z_in = nc.dram_tensor("z_in", [Cm, LH], bf16, kind="ExternalInput")
    xc_in = nc.dram_tensor("xc_in", [Cm, LH], bf16, kind="ExternalInput")
    dtot = nc.dram_tensor("dtot", [Cm, 1], f32, kind="ExternalInput")
    wfin = nc.dram_tensor("wfin", [Cm, C2], bf16, kind="ExternalInput")
    ones128 = nc.dram_tensor("ones128", [Cm, 1], f32, kind="ExternalInput")
    rrow = nc.dram_tensor("rrow", [1, LH], bf16, kind="Internal")
    mrow = nc.dram_tensor("mrow", [1, LH], bf16, kind="Internal")
    d_out = nc.dram_tensor("d_out", [C2, LH], bf16, kind="ExternalOutput")

    QL = LH // 128  # 64

    def bc_ap(t, sl):  # DRAM row slice -> partition-replicated AP
        return bass.AP(tensor=t, offset=sl.start, ap=[[0, 128], [1, sl.stop - sl.start]])

    with tile.TileContext(nc) as tc, \
         tc.tile_pool(name="w", bufs=1) as wp, \
         tc.tile_pool(name="d", bufs=1) as dp, \
         tc.tile_pool(name="c", bufs=2) as cp, \
         tc.tile_pool(name="rb", bufs=2) as bp, \
         tc.tile_pool(name="st", bufs=1) as sp, \
         tc.tile_pool(name="ps1", bufs=2, space="PSUM") as ps1, \
         tc.tile_pool(name="ps3", bufs=2, space="PSUM") as ps3:
        twa = wp.tile([Cm, 128], bf16)
        twb = wp.tile([Cm, 128], bf16)
        nc.sync.dma_start(out=twa, in_=wfin[:, 0:128])
        nc.sync.dma_start(out=twb, in_=wfin[:, 128:256])
        tone = wp.tile([Cm, 1], bf16)
        nc.gpsimd.dma_start(out=tone, in_=ones128[:, :])
        tdt = wp.tile([Cm, 1], f32)
        nc.scalar.dma_start(out=tdt, in_=dtot[:, :])
        teps = wp.tile([128, 1], f32)
        nc.vector.memset(teps, 1e-5)

        tm0 = dp.tile([Cm, LH], bf16)
        tm1 = dp.tile([Cm, LH], bf16)
        tz = dp.tile([Cm, LH], bf16)
        txc = dp.tile([Cm, LH], bf16)
        for ci in range(NC3):
            s = slice(ci * CH3, (ci + 1) * CH3)
            nc.sync.dma_start(out=tm0[:, s], in_=m02[:, s])
            nc.scalar.dma_start(out=tm1[:, s], in_=m13[:, s])
            nc.sync.dma_start(out=txc[:, s], in_=xc_in[:, s])
            nc.scalar.dma_start(out=tz[:, s], in_=z_in[:, s])

        ty = dp.tile([Cm, LH], bf16)        # y = m0 + m1 + Dtot*xc
        QC = CH3 // 128                     # 16 row-gather cols per chunk

        def pass1_rows(ci):
            sl = slice(ci * CH3, (ci + 1) * CH3)
            tdx = cp.tile([Cm, CH3], bf16, tag="dx")
            nc.vector.tensor_scalar_mul(out=tdx, in0=txc[:, sl],
                                        scalar1=tdt[:, 0:1])
            ta_ = cp.tile([Cm, CH3], bf16, tag="tya")
            nc.vector.tensor_tensor(out=ta_, in0=tm0[:, sl], in1=tm1[:, sl], op=ADD)
            nc.vector.tensor_tensor(out=ty[:, sl], in0=ta_, in1=tdx, op=ADD)
            tsq = cp.tile([Cm, CH3], bf16, tag="sq")
            nc.vector.tensor_tensor(out=tsq, in0=ty[:, sl], in1=ty[:, sl], op=MULT)
            trowc = cp.tile([33, CH3], bf16, tag="rw")
            for j in range(CH3 // 512):
                s2 = slice(j * 512, (j + 1) * 512)
                s2g = slice(ci * CH3 + j * 512, ci * CH3 + (j + 1) * 512)
                pst = ps1.tile([33, 512], f32, tag="st")
                nc.tensor.matmul(out=pst[0:1, :], lhsT=tone[:, :],
                                 rhs=ty[:, s2g], start=True, stop=True)
                nc.tensor.matmul(out=pst[32:33, :], lhsT=tone[:, :],
                                 rhs=tsq[:, s2], start=True, stop=True)
                nc.scalar.activation(out=trowc[:, s2], in_=pst[:, :],
                                     func=AF.Identity, bias=0.0, scale=1.0)

            tmu2 = cp.tile([128, QC], bf16, tag="r1")
            tss2 = cp.tile([128, QC], bf16, tag="r2")
            nc.sync.dma_start(out=tmu2[:, :], in_=trowc[0:1, :])
            nc.scalar.dma_start(out=tss2[:, :], in_=trowc[32:33, :])
            tvar = cp.tile([128, QC], f32, tag="r3")
            nc.vector.tensor_tensor(out=tvar, in0=tmu2, in1=tmu2, op=MULT)
            nc.vector.tensor_tensor(out=tvar, in0=tss2, in1=tvar, op=SUB)
            tlnv = cp.tile([128, QC], f32, tag="r4")
            nc.scalar.activation(out=tlnv, in_=tvar, func=AF.Ln,
                                 bias=teps[:, :], scale=1.0)
            trst = cp.tile([128, QC], bf16, tag="r5")
            nc.scalar.activation(out=trst, in_=tlnv, func=AF.Exp,
                                 bias=0.0, scale=-0.5)
            tmr = cp.tile([128, QC], bf16, tag="r6")
            nc.vector.tensor_tensor(out=tmr, in0=tmu2, in1=trst, op=MULT)
            nc.vector.tensor_scalar_mul(out=tmr, in0=tmr, scalar1=-1.0)
            nc.sync.dma_start(out=rrow[0:1, sl], in_=trst[:, :])
            nc.scalar.dma_start(out=mrow[0:1, sl], in_=tmr[:, :])

        def pass2(ci):
            # normalize + z-gate + final matmul for this chunk
            sl = slice(ci * CH3, (ci + 1) * CH3)
            tRb = bp.tile([Cm, CH3], bf16, tag="Rb")
            nc.scalar.dma_start(out=tRb, in_=bc_ap(rrow, sl))
            tSb = bp.tile([Cm, CH3], bf16, tag="Sb")
            nc.sync.dma_start(out=tSb, in_=bc_ap(mrow, sl))
            tt1 = cp.tile([Cm, CH3], bf16, tag="t1")
            nc.vector.tensor_tensor(out=tt1, in0=ty[:, sl], in1=tRb, op=MULT)
            tt2 = cp.tile([Cm, CH3], bf16, tag="t2")
            nc.vector.tensor_tensor(out=tt2, in0=tt1, in1=tSb, op=ADD)
            tt3 = cp.tile([Cm, CH3], bf16, tag="t3")
            nc.vector.tensor_tensor(out=tt3, in0=tt2, in1=tz[:, sl], op=MULT)
            for j in range(CH3 // 512):
                s2 = slice(j * 512, (j + 1) * 512)
                s2g = slice(ci * CH3 + j * 512, ci * CH3 + (j + 1) * 512)
                pd = ps3.tile([128, 1024], f32, tag="d")
                nc.tensor.matmul(out=pd[:, 0:512], lhsT=twa[:, :],
                                 rhs=tt3[:, s2], start=True, stop=True)
                nc.tensor.matmul(out=pd[:, 512:1024], lhsT=twb[:, :],
                                 rhs=tt3[:, s2], start=True, stop=True)
                td_ = cp.tile([128, 1024], bf16, tag="td")
                if (ci * 4 + j) % 3 == 2:
                    nc.vector.tensor_copy(out=td_, in_=pd[:, :])
                else:
                    nc.scalar.activation(out=td_, in_=pd[:, :],
                                         func=AF.Identity, bias=0.0, scale=1.0)
                nc.gpsimd.dma_start(out=d_out[0:128, s2g], in_=td_[:, 0:512])
                nc.sync.dma_start(out=d_out[128:256, s2g], in_=td_[:, 512:1024])

        # software pipeline: pass2 lags pass1 by one chunk so the engines never
        # sit on the bcast-row round trip
        for ci in range(NC3):
            pass1_rows(ci)
            if ci >= 1:
                pass2(ci - 1)
        pass2(NC3 - 1)
    return nc


# ------------------------------------------------------------------- host
def _get_ncs():
    if "ncs" not in _CACHE:
        nc1, nc2, nc3 = build_l1(), build_l2(), build_l3()
        for n in (nc1, nc2, nc3):
            _split_multiwaits(n)
        _CACHE["ncs"] = (nc1, nc2, nc3)
    return _CACHE["ncs"]


def kernel(x, cv1_w, cv1_b, scale_w, in_proj_w, conv_w, conv_b, x_proj_w,
           dt_w, dt_b, A_logs, Ds, ln_g, ln_b, out_proj_w, cv2_w, cv2_b):
    f = np.float32
    x = np.asarray(x, f)
    cv1_w = np.asarray(cv1_w, f); cv1_b = np.asarray(cv1_b, f)
    in_proj_w = np.asarray(in_proj_w, f)
    conv_w = np.asarray(conv_w, f); conv_b = np.asarray(conv_b, f)
    x_proj_w = np.asarray(x_proj_w, f)
    dt_w = np.asarray(dt_w, f); dt_b = np.asarray(dt_b, f)
    A_logs = np.asarray(A_logs, f); Ds = np.asarray(Ds, f)
    ln_g = np.asarray(ln_g, f); ln_b = np.asarray(ln_b, f)
    out_proj_w = np.asarray(out_proj_w, f)
    cv2_w = np.asarray(cv2_w, f); cv2_b = np.asarray(cv2_b, f)
    scale_v = np.asarray(scale_w, f).reshape(Cm)

    Wip_x, Wip_z = in_proj_w[:Cm], in_proj_w[Cm:]
    dwk = conv_w[:, 0]
    A = -np.exp(A_logs).reshape(K, Cm)
    Dk = Ds.reshape(K, Cm)
    Dtot = Dk.sum(axis=0)                              # (Cm,)
    Wdts8 = x_proj_w[:, :R]                            # (K, R, Cm)
    WB, WC = x_proj_w[:, R], x_proj_w[:, R + 1]        # (K, Cm)
    W_final = cv2_w @ (scale_v[:, None] * out_proj_w)
    W_final_g = W_final * ln_g[None, :]

    # fold lhsT: (tap, k=h-chan, m=out-chan) -> host layout (k, tap, m)
    Wfold = np.einsum('cyx,cd->yxdc', dwk, Wip_x)      # (3,3, in, out)
    wfold_rm = np.ascontiguousarray(
        Wfold.reshape(9, Cm, Cm).transpose(1, 0, 2))   # row-major cores

    nc1, nc2, nc3 = _get_ncs()

    # ---------------- L1 ----------------
    l1_maps = []
    for core in range(8):
        b, half = core // 2, core % 2
        r0 = half * HH
        xs = np.zeros((C1, HH + 2, W), NBF)
        lo, hi = r0 - 1, r0 + HH + 1
        slo, shi = max(lo, 0), min(hi, H)
        xs[:, slo - lo: shi - lo, :] = x[b, :, slo:shi, :].astype(NBF)
        mask = np.ones((Cm, 2), np.float32)
        mask[:, 0] = 0.0 if half == 0 else 1.0
        mask[:, 1] = 1.0 if half == 0 else 0.0
        l1_maps.append({
            "x_in": xs,
            "wcv1": np.ascontiguousarray(cv1_w.T).astype(NBF),
            "bcv1": cv1_b.reshape(Cm, 1),
            "wfold": wfold_rm.astype(NBF),
            "bconv": conv_b.reshape(Cm, 1),
            "wz": np.ascontiguousarray(Wip_z.T).astype(NBF),
            "hmask": mask,
        })
    r1 = _run(nc1, l1_maps, "L1")

    xc = np.zeros((B, Cm, L), NBF)
    zf = np.zeros((B, Cm, L), NBF)
    for core in range(8):
        b, half = core // 2, core % 2
        sl = slice(half * LH, (half + 1) * LH)
        xc[b][:, sl] = r1[core]["xc_out"]
        zf[b][:, sl] = r1[core]["z_out"]

    # ---------------- L2 ----------------
    def t_spatial(a):
        return np.ascontiguousarray(
            a.reshape(*a.shape[:-1], H, W).swapaxes(-1, -2).reshape(*a.shape[:-1], L))

    l2_maps = []
    for core in range(8):
        b, g = core // 2, core % 2
        if g == 0:
            u = xc[b]
            kf, kr = 0, 2
        else:
            u = t_spatial(xc[b])
            kf, kr = 1, 3
        uf = u.astype(np.float32)
        m = {}
        for sfx, k in (("f", kf), ("r", kr)):
            m["dts_" + sfx] = (Wdts8[k] @ uf).astype(NBF)
            brow = WB[k] @ uf
            m["crow_" + sfx] = (WC[k] @ uf).astype(NBF).reshape(1, L)
            m["v_" + sfx] = (uf * brow[None, :]).astype(NBF)
            m["wdt_" + sfx] = np.ascontiguousarray(dt_w[k].T).astype(NBF)  # (R, Cm)
            m["dtb_" + sfx] = dt_b[k].reshape(Cm, 1)
            m["a_" + sfx] = A[k].reshape(Cm, 1).astype(f)
        l2_maps.append(m)
    r2 = _run(nc2, l2_maps, "L2")

    # ---------------- L3 ----------------
    l3_maps = []
    for b in range(B):
        m02 = r2[2 * b]["m_out"]
        m13t = t_spatial(r2[2 * b + 1]["m_out"])
        for half in range(2):
            sl = slice(half * LH, (half + 1) * LH)
            l3_maps.append({
                "m02": np.ascontiguousarray(m02[:, sl]),
                "m13": np.ascontiguousarray(m13t[:, sl]),
                "z_in": np.ascontiguousarray(zf[b][:, sl]),
                "xc_in": np.ascontiguousarray(xc[b][:, sl]),
                "dtot": Dtot.reshape(Cm, 1),
                "wfin": np.ascontiguousarray(W_final_g.T).astype(NBF),
                "ones128": np.full((Cm, 1), 1.0 / Cm, np.float32),
            })
    r3 = _run(nc3, l3_maps, "L3")

    out = np.empty((B, C2, H, W), np.float32)
    for core in range(8):
        b, half = core // 2, core % 2
        sl = slice(half * LH, (half + 1) * LH)
        out[b].reshape(C2, L)[:, sl] = r3[core]["d_out"]
    if np.any(ln_b != 0.0):
        # ln beta term, folded past the z-gate on the host: W_final @ (b .* z)
        Wb = W_final * ln_b[None, :]
        for b in range(B):
            out[b].reshape(C2, L)[:, :] += (Wb @ zf[b].astype(np.float32))
    out += x
    out += cv2_b[None, :, None, None]
    return out


# revision 28
# speedup vs baseline: 1.2918x; 1.0016x over previous
"""BottleneckMamba Trainium2 kernel (self-contained), v2.

out = x + cv2( scale * out_proj( LN(cross-merge(4-dir selective scan(N=1))) * z ) )

3 SPMD launches on 8 NeuronCores:
  L1 (core=(b, image-half)): cv1 -> h (bias on DVE); depthwise3x3*in_proj
     folded into 9 matmuls -> silu -> xc ; z = silu(Wz@h).
  L2 (core=(b, dir-group)): per direction: dtd = dt_w8 @ dts (rank-8 rows
     preshipped) -> exp/ln1p/exp on ACT (one table set) -> tbt = dt*v
     (v = u*B preshipped) -> tensor_tensor_scan -> mc = h*C (C broadcast by
     replicating DMA). Chunks processed ping-pong (reverse dir from the top,
     forward from the bottom) so both scans stream concurrently; m = mcf+mcr.
  L3 (core=(b, half)): y = m02 + m13^T + (sum_k D_k).xc; LayerNorm stats via
     ones-matmul per position chunk (no global barrier: stats are
     per-position), rstd/-mu*rstd rows written to DRAM and replicated back by
     DMA, normalize * z, fused (cv2 @ diag(scale) @ out_proj @ diag(ln_g))
     matmul -> bf16 delta.
Host: shards/reassembles, transposes between launches, computes the rank-8
dt rows + B/C rows + v = u*B from xc (bf16 data prep), adds residual x,
cv2 bias and the (zero here) ln_b term.
"""
import os
import sys

sys.path.insert(0, '/opt/trn_rl_repo')

import numpy as np
import ml_dtypes

import concourse.bass as bass
import concourse.tile as tile
import concourse.mybir as mybir
from concourse.bass_utils import run_bass_kernel_spmd

bf16 = mybir.dt.bfloat16
f32 = mybir.dt.float32
MULT, ADD = mybir.AluOpType.mult, mybir.AluOpType.add
SUB = mybir.AluOpType.subtract
AF = mybir.ActivationFunctionType
NBF = ml_dtypes.bfloat16

B, C1, C2, H, W = 4, 256, 256, 128, 128
Cm, K, R = 128, 4, 8
L = H * W          # 16384
HH = H // 2        # 64 rows per half
LH = HH * W        # 8192
CH = 2048          # L2 chunk
NCH = L // CH      # 8
CH3 = 2048         # L3 chunk
NC3 = LH // CH3    # 4

EXEC_TIMES = {}    # launch -> exec ns (MAMBA_TRACE=1)
TRACES = {}        # launch -> (insts, trace_path) (MAMBA_TRACE=1)
_CACHE = {}


def _split_multiwaits(nc):
    """walrus here accepts ONE sync-wait per instruction; hoist extras into
    single-wait same-engine NOPs inserted before the instruction."""
    for f in nc.m.functions:
        for bb in f.blocks:
            il = bb.instructions
            i = 0
            while i < len(il):
                ins = il[i]
                si = getattr(ins, "sync_info", None)
                if si is not None and len(si.on_wait) > 1:
                    waits = list(si.on_wait)
                    ins.sync_info = mybir.SyncInfo(
                        on_wait=[waits[-1]], on_update=list(si.on_update))
                    for w in waits[:-1]:
                        nop = mybir.InstNoOp(
                            name=nc.get_next_instruction_name(), ins=[], outs=[])
                        nop.engine = ins.engine
                        nop.sync_info = mybir.SyncInfo(on_wait=[w], on_update=[])
                        nc.register_instruction(nop, overwrite=True)
                        il.insert(i, nop)
                        i += 1
                i += 1


def _new_nc():
    return bass.Bass("TRN2", target_bir_lowering=False, debug=False,
                     enable_asserts=True, num_devices=8)


def _run(nc, in_maps, name):
    trace = os.environ.get("MAMBA_TRACE", "0") == "1"
    res = run_bass_kernel_spmd(nc, in_maps, core_ids=list(range(8)), trace=trace)
    if trace:
        EXEC_TIMES[name] = res.exec_time_ns
        TRACES[name] = res.instructions_and_trace
    return res.results


# ------------------------------------------------------------------- L1
def build_l1():
    nc = _new_nc()
    x_in = nc.dram_tensor("x_in", [C1, HH + 2, W], bf16, kind="ExternalInput")
    wcv1 = nc.dram_tensor("wcv1", [C1, Cm], bf16, kind="ExternalInput")      # lhsT
    bcv1 = nc.dram_tensor("bcv1", [Cm, 1], f32, kind="ExternalInput")
    wfold = nc.dram_tensor("wfold", [Cm, 9, Cm], bf16, kind="ExternalInput")  # (k, tap, m)
    bconv = nc.dram_tensor("bconv", [Cm, 1], f32, kind="ExternalInput")
    wz = nc.dram_tensor("wz", [Cm, Cm], bf16, kind="ExternalInput")          # lhsT
    hmask = nc.dram_tensor("hmask", [Cm, 2], f32, kind="ExternalInput")
    xc_out = nc.dram_tensor("xc_out", [Cm, LH], bf16, kind="ExternalOutput")
    z_out = nc.dram_tensor("z_out", [Cm, LH], bf16, kind="ExternalOutput")

    HP = HH + 2   # 66
    WP = W + 2    # 130

    with tile.TileContext(nc) as tc, \
         tc.tile_pool(name="w", bufs=1) as wp, \
         tc.tile_pool(name="d", bufs=1) as dp, \
         tc.tile_pool(name="ps", bufs=2, space="PSUM") as pp:
        tw1a = wp.tile([128, Cm], bf16)
        tw1b = wp.tile([128, Cm], bf16)
        nc.sync.dma_start(out=tw1a, in_=wcv1[0:128, :])
        nc.scalar.dma_start(out=tw1b, in_=wcv1[128:256, :])
        tb1 = wp.tile([Cm, 1], f32)
        nc.scalar.dma_start(out=tb1, in_=bcv1[:, :])
        tbc = wp.tile([Cm, 1], f32)
        nc.scalar.dma_start(out=tbc, in_=bconv[:, :])
        tmask = wp.tile([Cm, 2], f32)
        nc.scalar.dma_start(out=tmask, in_=hmask[:, :])

        # x as separate per-8-row-block tiles so each cv1 chunk gates only on
        # its own block's DMA, not the whole x load
        xblocks = [(rb, min(8, HP - rb)) for rb in range(0, HP, 8)]
        txa = [dp.tile([128, nr, W], bf16, name=f"txa{i}")
               for i, (_, nr) in enumerate(xblocks)]
        txb = [dp.tile([128, nr, W], bf16, name=f"txb{i}")
               for i, (_, nr) in enumerate(xblocks)]
        twf = wp.tile([Cm, 9, Cm], bf16)
        twz = wp.tile([Cm, Cm], bf16)
        for i, (rb, nr) in enumerate(xblocks):
            nc.sync.dma_start(out=txa[i], in_=x_in[0:128, rb:rb + nr, :])
            nc.scalar.dma_start(out=txb[i], in_=x_in[128:256, rb:rb + nr, :])
            if i == 1:  # fold/z weights after the first two x blocks
                nc.scalar.dma_start(out=twf, in_=wfold[:, :, :])
                nc.scalar.dma_start(out=twz, in_=wz[:, :])

        th = dp.tile([Cm, HP, WP], bf16)
        nc.vector.memset(th[:, :, 0:1], 0.0)
        nc.vector.memset(th[:, :, WP - 1:WP], 0.0)

        # cv1 over 66 rows: 16 chunks of 4 rows + 1 chunk of 2 rows.
        # Interleave cv1 with the fold chunks in PE issue order so the PE is
        # not queued behind the tail of the x load.
        row_chunks = [(r0, 4) for r0 in range(0, 64, 4)] + [(64, 2)]

        def cv1_chunk(idx):
            r0, nr = row_chunks[idx]
            blk, off = r0 // 8, r0 % 8
            pt = pp.tile([Cm, 512], f32, tag="cv1")
            nn = nr * W
            nc.tensor.matmul(out=pt[:, :nn], lhsT=tw1a[:, :],
                             rhs=txa[blk][:, off:off + nr, :], start=True, stop=False)
            nc.tensor.matmul(out=pt[:, :nn], lhsT=tw1b[:, :],
                             rhs=txb[blk][:, off:off + nr, :], start=False, stop=True)
            nc.vector.tensor_scalar_add(out=th[:, r0:r0 + nr, 1:W + 1],
                                        in0=pt[:, :nn], scalar1=tb1[:, 0:1])
            if idx == 0:
                nc.vector.tensor_scalar_mul(out=th[:, 0, :], in0=th[:, 0, :],
                                            scalar1=tmask[:, 0:1])
            if idx == len(row_chunks) - 1:
                nc.vector.tensor_scalar_mul(out=th[:, HP - 1, :],
                                            in0=th[:, HP - 1, :],
                                            scalar1=tmask[:, 1:2])

        cv1_chunk(0)
        cv1_chunk(1)

        txc = dp.tile([Cm, HH, W], bf16)
        tz = dp.tile([Cm, HH, W], bf16)
        for k in range(16):
            r0 = 4 * k
            pt = pp.tile([Cm, 512], f32, tag="fold")
            for t in range(9):
                dy, dx = t // 3 - 1, t % 3 - 1
                nc.tensor.matmul(
                    out=pt[:, :], lhsT=twf[:, t, :],
                    rhs=th[:, r0 + 1 + dy:r0 + 5 + dy, 1 + dx:W + 1 + dx],
                    start=(t == 0), stop=(t == 8))
            nc.scalar.activation(out=txc[:, r0:r0 + 4, :], in_=pt[:, :],
                                 func=AF.Silu, bias=tbc[:, :], scale=1.0)
            ptz = pp.tile([Cm, 512], f32, tag="z")
            nc.tensor.matmul(out=ptz[:, :], lhsT=twz[:, :],
                             rhs=th[:, r0 + 1:r0 + 5, 1:W + 1],
                             start=True, stop=True)
            nc.scalar.activation(out=tz[:, r0:r0 + 4, :], in_=ptz[:, :],
                                 func=AF.Silu, bias=0.0, scale=1.0)
            if k + 2 < len(row_chunks):
                cv1_chunk(k + 2)

            if r0 % 8 == 4:  # flush every 8 rows
                rs = r0 - 4
                nc.gpsimd.dma_start(out=xc_out[:, rs * W:(r0 + 4) * W],
                                    in_=txc[:, rs:r0 + 4, :])
                nc.gpsimd.dma_start(out=z_out[:, rs * W:(r0 + 4) * W],
                                  in_=tz[:, rs:r0 + 4, :])
    return nc


# ------------------------------------------------------------------- L2
def build_l2():
    nc = _new_nc()
    dts_f = nc.dram_tensor("dts_f", [R, L], bf16, kind="ExternalInput")
    dts_r = nc.dram_tensor("dts_r", [R, L], bf16, kind="ExternalInput")
    v_f = nc.dram_tensor("v_f", [Cm, L], bf16, kind="ExternalInput")
    v_r = nc.dram_tensor("v_r", [Cm, L], bf16, kind="ExternalInput")
    crow_f = nc.dram_tensor("crow_f", [1, L], bf16, kind="ExternalInput")
    crow_r = nc.dram_tensor("crow_r", [1, L], bf16, kind="ExternalInput")
    wdt_f = nc.dram_tensor("wdt_f", [R, Cm], bf16, kind="ExternalInput")  # lhsT
    wdt_r = nc.dram_tensor("wdt_r", [R, Cm], bf16, kind="ExternalInput")
    dtb_f = nc.dram_tensor("dtb_f", [Cm, 1], f32, kind="ExternalInput")
    dtb_r = nc.dram_tensor("dtb_r", [Cm, 1], f32, kind="ExternalInput")
    a_f = nc.dram_tensor("a_f", [Cm, 1], f32, kind="ExternalInput")
    a_r = nc.dram_tensor("a_r", [Cm, 1], f32, kind="ExternalInput")
    m_out = nc.dram_tensor("m_out", [Cm, L], bf16, kind="ExternalOutput")

    def bc_ap(t, sl):  # DRAM row slice -> partition-replicated AP
        return bass.AP(tensor=t, offset=sl.start, ap=[[0, 128], [1, sl.stop - sl.start]])

    with tile.TileContext(nc) as tc, \
         tc.tile_pool(name="w", bufs=1) as wp, \
         tc.tile_pool(name="full", bufs=1) as fp, \
         tc.tile_pool(name="ds", bufs=2) as dsp, \
         tc.tile_pool(name="vv", bufs=3) as vp, \
         tc.tile_pool(name="ck", bufs=3) as cp, \
         tc.tile_pool(name="hk", bufs=2) as hp, \
         tc.tile_pool(name="bc", bufs=3) as bp, \
         tc.tile_pool(name="mm", bufs=2) as mp, \
         tc.tile_pool(name="psd", bufs=2, space="PSUM") as psd:
        twf_ = wp.tile([R, Cm], bf16)
        twr_ = wp.tile([R, Cm], bf16)
        nc.sync.dma_start(out=twr_, in_=wdt_r[:, :])
        nc.sync.dma_start(out=twf_, in_=wdt_f[:, :])
        tbf = wp.tile([Cm, 1], f32)
        tbr = wp.tile([Cm, 1], f32)
        taf = wp.tile([Cm, 1], f32)
        tar = wp.tile([Cm, 1], f32)
        nc.scalar.dma_start(out=tbf, in_=dtb_f[:, :])
        nc.scalar.dma_start(out=tbr, in_=dtb_r[:, :])
        nc.scalar.dma_start(out=taf, in_=a_f[:, :])
        nc.scalar.dma_start(out=tar, in_=a_r[:, :])

        tmcf = fp.tile([Cm, L], bf16)   # h_f * C_f, natural position order
        tmcr = fp.tile([Cm, L], bf16)   # h_r * C_r, natural position order

        state = {"prev_f": None, "prev_r": None}

        def side(ci, rev, split=False):
            sl = slice(ci * CH, (ci + 1) * CH)
            sfx = "r" if rev else "f"
            tdts, tv_d, tcr, tw, tb, ta = (
                (dts_r, v_r, crow_r, twr_, tbr, tar) if rev else
                (dts_f, v_f, crow_f, twf_, tbf, taf))
            td = dsp.tile([R, CH], bf16, tag="d" + sfx)
            nc.gpsimd.dma_start(out=td, in_=tdts[:, sl])
            tv = vp.tile([Cm, CH], bf16, tag="v" + sfx)
            nc.sync.dma_start(out=tv, in_=tv_d[:, sl])
            tcb = bp.tile([Cm, CH], bf16, tag="c" + sfx)
            nc.gpsimd.dma_start(out=tcb, in_=bc_ap(tcr, sl))

            pt = psd.tile([Cm, CH], f32, tag="dtd")
            for j in range(CH // 512):
                nc.tensor.matmul(
                    out=pt[:, j * 512:(j + 1) * 512], lhsT=tw[:, :],
                    rhs=td[:, j * 512:(j + 1) * 512], start=True, stop=True)
            te1 = cp.tile([Cm, CH], bf16, tag="e1")
            tdt = cp.tile([Cm, CH], bf16, tag="dt")
            tav = cp.tile([Cm, CH], bf16, tag="av")
            tbt = cp.tile([Cm, CH], bf16, tag="bt")
            thc = hp.tile([Cm, CH], bf16, tag="h" + sfx)
            # For the very first chunk, run the chain in halves so the first
            # scan starts ~3us earlier (reverse dir: high half scans first).
            halves = ([slice(CH // 2, CH), slice(0, CH // 2)] if split
                      else [slice(0, CH)])
            for hs in halves:
                nc.scalar.activation(out=te1[:, hs], in_=pt[:, hs], func=AF.Exp,
                                     bias=tb[:, :], scale=1.0)
                nc.scalar.activation(out=tdt[:, hs], in_=te1[:, hs], func=AF.Ln,
                                     bias=1.0, scale=1.0)
                nc.scalar.activation(out=tav[:, hs], in_=tdt[:, hs], func=AF.Exp,
                                     bias=0.0, scale=ta[:, :])
                nc.vector.tensor_tensor(out=tbt[:, hs], in0=tdt[:, hs],
                                        in1=tv[:, hs], op=MULT)
                if rev:
                    prev = state["prev_r"]
                    nc.vector.tensor_tensor_scan(
                        out=thc[:, hs][:, ::-1], data0=tav[:, hs][:, ::-1],
                        data1=tbt[:, hs][:, ::-1],
                        initial=0.0 if prev is None else prev,
                        op0=MULT, op1=ADD)
                    state["prev_r"] = thc[:, hs.start:hs.start + 1]
                else:
                    prev = state["prev_f"]
                    nc.vector.tensor_tensor_scan(
                        out=thc[:, hs], data0=tav[:, hs], data1=tbt[:, hs],
                        initial=0.0 if prev is None else prev,
                        op0=MULT, op1=ADD)
                    state["prev_f"] = thc[:, hs.stop - 1:hs.stop]
            if rev:
                nc.vector.tensor_tensor(out=tmcr[:, sl], in0=thc, in1=tcb, op=MULT)
            else:
                nc.vector.tensor_tensor(out=tmcf[:, sl], in0=thc, in1=tcb, op=MULT)

        for s in range(NCH):
            side(NCH - 1 - s, rev=True, split=(s == 0))
            side(s, rev=False)
            if s >= NCH // 2:
                for c in (s, NCH - 1 - s):
                    slc = slice(c * CH, (c + 1) * CH)
                    tm = mp.tile([Cm, CH], bf16, tag="m")
                    nc.vector.tensor_tensor(out=tm, in0=tmcf[:, slc],
                                            in1=tmcr[:, slc], op=ADD)
                    nc.scalar.dma_start(out=m_out[:, slc], in_=tm)
    return nc


# ------------------------------------------------------------------- L3
def build_l3():
    nc = _new_nc()
    m02 = nc.dram_tensor("m02", [Cm, LH], bf16, kind="ExternalInput")
    m13 = nc.dram_tensor("m13", [Cm, LH], bf16, kind="ExternalInput")
    z_in = nc.dram_tensor("z_in", [Cm, LH], bf16, kind="ExternalInput")
    xc_in = nc.dram_tensor("xc_in", [Cm, LH], bf16, kind="ExternalInput")
    dtot = nc.dram_tensor("dtot", [Cm, 1], f32, kind="ExternalInput")
    wfin = nc.dram_tensor("wfin", [Cm, C2], bf16, kind="ExternalInput")
    ones128 = nc.dram_tensor("ones128", [Cm, 1], f32, kind="ExternalInput")
    rrow = nc.dram_tensor("rrow", [1, LH], bf16, kind="Internal")
    mrow = nc.dram_tensor("mrow", [1, LH], bf16, kind="Internal")
    d_out = nc.dram_tensor("d_out", [C2, LH], bf16, kind="ExternalOutput")

    QL = LH // 128  # 64

    def bc_ap(t, sl):  # DRAM row slice -> partition-replicated AP
        return bass.AP(tensor=t, offset=sl.start, ap=[[0, 128], [1, sl.stop - sl.start]])

    with tile.TileContext(nc) as tc, \
         tc.tile_pool(name="w", bufs=1) as wp, \
         tc.tile_pool(name="d", bufs=1) as dp, \
         tc.tile_pool(name="c", bufs=2) as cp, \
         tc.tile_pool(name="rb", bufs=2) as bp, \
         tc.tile_pool(name="st", bufs=1) as sp, \
         tc.tile_pool(name="ps1", bufs=2, space="PSUM") as ps1, \
         tc.tile_pool(name="ps3", bufs=2, space="PSUM") as ps3:
        twa = wp.tile([Cm, 128], bf16)
        twb = wp.tile([Cm, 128], bf16)
        nc.sync.dma_start(out=twa, in_=wfin[:, 0:128])
        nc.sync.dma_start(out=twb, in_=wfin[:, 128:256])
        tone = wp.tile([Cm, 1], bf16)
        nc.gpsimd.dma_start(out=tone, in_=ones128[:, :])
        tdt = wp.tile([Cm, 1], f32)
        nc.scalar.dma_start(out=tdt, in_=dtot[:, :])
        teps = wp.tile([128, 1], f32)
        nc.vector.memset(teps, 1e-5)

        tm0 = dp.tile([Cm, LH], bf16)
        tm1 = dp.tile([Cm, LH], bf16)
        tz = dp.tile([Cm, LH], bf16)
        txc = dp.tile([Cm, LH], bf16)
        for ci in range(NC3):
            s = slice(ci * CH3, (ci + 1) * CH3)
            nc.sync.dma_start(out=tm0[:, s], in_=m02[:, s])
            nc.scalar.dma_start(out=tm1[:, s], in_=m13[:, s])
            nc.sync.dma_start(out=txc[:, s], in_=xc_in[:, s])
            nc.scalar.dma_start(out=tz[:, s], in_=z_in[:, s])

        ty = dp.tile([Cm, LH], bf16)        # y = m0 + m1 + Dtot*xc
        QC = CH3 // 128                     # 16 row-gather cols per chunk

        def pass1_rows(ci):
            sl = slice(ci * CH3, (ci + 1) * CH3)
            tdx = cp.tile([Cm, CH3], bf16, tag="dx")
            nc.vector.tensor_scalar_mul(out=tdx, in0=txc[:, sl],
                                        scalar1=tdt[:, 0:1])
            ta_ = cp.tile([Cm, CH3], bf16, tag="tya")
            nc.vector.tensor_tensor(out=ta_, in0=tm0[:, sl], in1=tm1[:, sl], op=ADD)
            nc.vector.tensor_tensor(out=ty[:, sl], in0=ta_, in1=tdx, op=ADD)
            tsq = cp.tile([Cm, CH3], bf16, tag="sq")
            nc.vector.tensor_tensor(out=tsq, in0=ty[:, sl], in1=ty[:, sl], op=MULT)
            trowc = cp.tile([33, CH3], bf16, tag="rw")
            for j in range(CH3 // 512):
                s2 = slice(j * 512, (j + 1) * 512)
                s2g = slice(ci * CH3 + j * 512, ci * CH3 + (j + 1) * 512)
                pst = ps1.tile([33, 512], f32, tag="st")
                nc.tensor.matmul(out=pst[0:1, :], lhsT=tone[:, :],
                                 rhs=ty[:, s2g], start=True, stop=True)
                nc.tensor.matmul(out=pst[32:33, :], lhsT=tone[:, :],
                                 rhs=tsq[:, s2], start=True, stop=True)
                nc.scalar.activation(out=trowc[:, s2], in_=pst[:, :],
                                     func=AF.Identity, bias=0.0, scale=1.0)

            tmu2 = cp.tile([128, QC], bf16, tag="r1")
            tss2 = cp.tile([128, QC], bf16, tag="r2")
            nc.sync.dma_start(out=tmu2[:, :], in_=trowc[0:1, :])
            nc.scalar.dma_start(out=tss2[:, :], in_=trowc[32:33, :])
            tvar = cp.tile([128, QC], f32, tag="r3")
            nc.vector.tensor_tensor(out=tvar, in0=tmu2, in1=tmu2, op=MULT)
            nc.vector.tensor_tensor(out=tvar, in0=tss2, in1=tvar, op=SUB)
            tlnv = cp.tile([128, QC], f32, tag="r4")
            nc.scalar.activation(out=tlnv, in_=tvar, func=AF.Ln,
                                 bias=teps[:, :], scale=1.0)
            trst = cp.tile([128, QC], bf16, tag="r5")
            nc.scalar.activation(out=trst, in_=tlnv, func=AF.Exp,
                                 bias=0.0, scale=-0.5)
            tmr = cp.tile([128, QC], bf16, tag="r6")
            nc.vector.tensor_tensor(out=tmr, in0=tmu2, in1=trst, op=MULT)
            nc.vector.tensor_scalar_mul(out=tmr, in0=tmr, scalar1=-1.0)
            nc.sync.dma_start(out=rrow[0:1, sl], in_=trst[:, :])
            nc.scalar.dma_start(out=mrow[0:1, sl], in_=tmr[:, :])

        def pass2(ci):
            # normalize + z-gate + final matmul for this chunk
            sl = slice(ci * CH3, (ci + 1) * CH3)
            tRb = bp.tile([Cm, CH3], bf16, tag="Rb")
            nc.scalar.dma_start(out=tRb, in_=bc_ap(rrow, sl))
            tSb = bp.tile([Cm, CH3], bf16, tag="Sb")
            nc.sync.dma_start(out=tSb, in_=bc_ap(mrow, sl))
            tt1 = cp.tile([Cm, CH3], bf16, tag="t1")
            nc.vector.tensor_tensor(out=tt1, in0=ty[:, sl], in1=tRb, op=MULT)
            tt2 = cp.tile([Cm, CH3], bf16, tag="t2")
            nc.vector.tensor_tensor(out=tt2, in0=tt1, in1=tSb, op=ADD)
            tt3 = cp.tile([Cm, CH3], bf16, tag="t3")
            nc.vector.tensor_tensor(out=tt3, in0=tt2, in1=tz[:, sl], op=MULT)
            for j in range(CH3 // 512):
                s2 = slice(j * 512, (j + 1) * 512)
                s2g = slice(ci * CH3 + j * 512, ci * CH3 + (j + 1) * 512)
                pd = ps3.tile([128, 1024], f32, tag="d")
                nc.tensor.matmul(out=pd[:, 0:512], lhsT=twa[:, :],
                                 rhs=tt3[:, s2], start=True, stop=True)
                nc.tensor.matmul(out=pd[:, 512:1024], lhsT=twb[:, :],
                                 rhs=tt3[:, s2], start=True, stop=True)
                td_ = cp.tile([128, 1024], bf16, tag="td")
                if (ci * 4 + j) % 3 == 2:
                    nc.vector.tensor_copy(out=td_, in_=pd[:, :])
                else:
                    nc.scalar.activation(out=td_, in_=pd[:, :],
                                         func=AF.Identity, bias=0.0, scale=1.0)
                nc.gpsimd.dma_start(out=d_out[0:128, s2g], in_=td_[:, 0:512])
                nc.sync.dma_start(out=d_out[128:256, s2g], in_=td_[:, 512:1024])

        # software pipeline: pass2 lags pass1 by one chunk so the engines never
        # sit on the bcast-row round trip
        for ci in range(NC3):
            pass1_rows(ci)
            if ci >= 1:
                pass2(ci - 1)
        pass2(NC3 - 1)
    return nc


# ------------------------------------------------------------------- host
def _get_ncs():
    if "ncs" not in _CACHE:
        nc1, nc2, nc3 = build_l1(), build_l2(), build_l3()
        for n in (nc1, nc2, nc3):
            _split_multiwaits(n)
        _CACHE["ncs"] = (nc1, nc2, nc3)
    return _CACHE["ncs"]


def kernel(x, cv1_w, cv1_b, scale_w, in_proj_w, conv_w, conv_b, x_proj_w,
           dt_w, dt_b, A_logs, Ds, ln_g, ln_b, out_proj_w, cv2_w, cv2_b):
    f = np.float32
    x = np.asarray(x, f)
    cv1_w = np.asarray(cv1_w, f); cv1_b = np.asarray(cv1_b, f)
    in_proj_w = np.asarray(in_proj_w, f)
    conv_w = np.asarray(conv_w, f); conv_b = np.asarray(conv_b, f)
    x_proj_w = np.asarray(x_proj_w, f)
    dt_w = np.asarray(dt_w, f); dt_b = np.asarray(dt_b, f)
    A_logs = np.asarray(A_logs, f); Ds = np.asarray(Ds, f)
    ln_g = np.asarray(ln_g, f); ln_b = np.asarray(ln_b, f)
    out_proj_w = np.asarray(out_proj_w, f)
    cv2_w = np.asarray(cv2_w, f); cv2_b = np.asarray(cv2_b, f)
    scale_v = np.asarray(scale_w, f).reshape(Cm)

    Wip_x, Wip_z = in_proj_w[:Cm], in_proj_w[Cm:]
    dwk = conv_w[:, 0]
    A = -np.exp(A_logs).reshape(K, Cm)
    Dk = Ds.reshape(K, Cm)
    Dtot = Dk.sum(axis=0)                              # (Cm,)
    Wdts8 = x_proj_w[:, :R]                            # (K, R, Cm)
    WB, WC = x_proj_w[:, R], x_proj_w[:, R + 1]        # (K, Cm)
    W_final = cv2_w @ (scale_v[:, None] * out_proj_w)
    W_final_g = W_final * ln_g[None, :]

    # fold lhsT: (tap, k=h-chan, m=out-chan) -> host layout (k, tap, m)
    Wfold = np.einsum('cyx,cd->yxdc', dwk, Wip_x)      # (3,3, in, out)
    wfold_rm = np.ascontiguousarray(
        Wfold.reshape(9, Cm, Cm).transpose(1, 0, 2))   # row-major cores

    nc1, nc2, nc3 = _get_ncs()

    # ---------------- L1 ----------------
    l1_maps = []
    for core in range(8):
        b, half = core // 2, core % 2
        r0 = half * HH
        xs = np.zeros((C1, HH + 2, W), NBF)
        lo, hi = r0 - 1, r0 + HH + 1
        slo, shi = max(lo, 0), min(hi, H)
        xs[:, slo - lo: shi - lo, :] = x[b, :, slo:shi, :].astype(NBF)
        mask = np.ones((Cm, 2), np.float32)
        mask[:, 0] = 0.0 if half == 0 else 1.0
        mask[:, 1] = 1.0 if half == 0 else 0.0
        l1_maps.append({
            "x_in": xs,
            "wcv1": np.ascontiguousarray(cv1_w.T).astype(NBF),
            "bcv1": cv1_b.reshape(Cm, 1),
            "wfold": wfold_rm.astype(NBF),
            "bconv": conv_b.reshape(Cm, 1),
            "wz": np.ascontiguousarray(Wip_z.T).astype(NBF),
            "hmask": mask,
        })
    r1 = _run(nc1, l1_maps, "L1")

    xc = np.zeros((B, Cm, L), NBF)
    zf = np.zeros((B, Cm, L), NBF)
    for core in range(8):
        b, half = core // 2, core % 2
        sl = slice(half * LH, (half + 1) * LH)
        xc[b][:, sl] = r1[core]["xc_out"]
        zf[b][:, sl] = r1[core]["z_out"]

    # ---------------- L2 ----------------
    def t_spatial(a):
        return np.ascontiguousarray(
            a.reshape(*a.shape[:-1], H, W).swapaxes(-1, -2).reshape(*a.shape[:-1], L))

    l2_maps = []
    for core in range(8):
        b, g = core // 2, core % 2
        if g == 0:
            u = xc[b]
            kf, kr = 0, 2
        else:
            u = t_spatial(xc[b])
            kf, kr = 1, 3
        uf = u.astype(np.float32)
        m = {}
        for sfx, k in (("f", kf), ("r", kr)):
            m["dts_" + sfx] = (Wdts8[k] @ uf).astype(NBF)
            brow = WB[k] @ uf
            m["crow_" + sfx] = (WC[k] @ uf).astype(NBF).reshape(1, L)
            m["v_" + sfx] = (uf * brow[None, :]).astype(NBF)
            m["wdt_" + sfx] = np.ascontiguousarray(dt_w[k].T).astype(NBF)  # (R, Cm)
            m["dtb_" + sfx] = dt_b[k].reshape(Cm, 1)
            m["a_" + sfx] = A[k].reshape(Cm, 1).astype(f)
        l2_maps.append(m)
    r2 = _run(nc2, l2_maps, "L2")

    # ---------------- L3 ----------------
    l3_maps = []
    for b in range(B):
        m02 = r2[2 * b]["m_out"]
        m13t = t_spatial(r2[2 * b + 1]["m_out"])
        for half in range(2):
            sl = slice(half * LH, (half + 1) * LH)
            l3_maps.append({
                "m02": np.ascontiguousarray(m02[:, sl]),
                "m13": np.ascontiguousarray(m13t[:, sl]),
                "z_in": np.ascontiguousarray(zf[b][:, sl]),
                "xc_in": np.ascontiguousarray(xc[b][:, sl]),
                "dtot": Dtot.reshape(Cm, 1),
                "wfin": np.ascontiguousarray(W_final_g.T).astype(NBF),
                "ones128": np.full((Cm, 1), 1.0 / Cm, np.float32),
            })
    r3 = _run(nc3, l3_maps, "L3")

    out = np.empty((B, C2, H, W), np.float32)
    for core in range(8):
        b, half = core // 2, core % 2
        sl = slice(half * LH, (half + 1) * LH)
        out[b].reshape(C2, L)[:, sl] = r3[core]["d_out"]
    if np.any(ln_b != 0.0):
        # ln beta term, folded past the z-gate on the host: W_final @ (b .* z)
        Wb = W_final * ln_b[None, :]
        for b in range(B):
            out[b].reshape(C2, L)[:, :] += (Wb @ zf[b].astype(np.float32))
    out += x
    out += cv2_b[None, :, None, None]
    return out
